# revision 28
# baseline (speedup 1.0000x reference)
"""Trainium2 Bass kernel for DualConsistencyRegularization.

Contract: kernel(**inputs) takes FULL unsharded inputs (B=64) and returns
(out_d, out_n, out_e, reg_loss) matching reference.reference(**inputs).

Strategy: pure data parallel over batch (8 per core x 8 cores). Each core
runs an identical NEFF on its batch shard; reg_loss partials are summed on
host (mean over batch == sum of per-core partial sums / B).

Kernel structure per core, per stream (d/n/e):
 - P1 token LN + segment pooling: per 128-token tile, token mean (DVE
   reduce) and sum-of-squares (ACT Square accum) feed rsqrt(var+eps); the
   pooling matmul contracts tokens with the per-token r baked into a mask
   (1/L also baked), with the token means as rhs column 256 so the mean
   correction comes out as psum column 256. float32r keeps PE at full rate.
 - MLP/GCN/proj run feature-on-partitions at [128, 240] (8 batches x 30
   segments); LN over the partition dim uses ones-matmul stats + K=1
   broadcast matmuls. Softmax row sums make A.sum == 2 exactly, so
   normalization is (softmax + I)/2, with the 1/2 folded into the msg copy.
 - Scatter+blend: one-hot mask matmul gathers proj rows per token tile
   ((1-alpha) baked into the mask); blend alternates between a fused DVE
   scalar_tensor_tensor and a PE alpha*I accumulate + ACT copy to balance
   engines.
"""

import numpy as np

B, S, D = 64, 915, 256
K, H = 30, 128
L = 30
BASE = 870
GAMMA, LAM, ALPHA = 0.1, 0.5, 0.9
EPS = 1e-5
NCORES = 8
BL = B // NCORES  # 8 batches per core
NT = 8            # token tiles per batch: 7 full x128 + 1 x19

PARAM_1D = [
    'ln_d_w', 'ln_d_b', 'ln_n_w', 'ln_n_b', 'ln_e_w', 'ln_e_b',
    'mlp_b1', 'mlp_ln1_w', 'mlp_ln1_b', 'mlp_b2', 'mlp_ln2_w', 'mlp_ln2_b',
    'gcn_ln_w', 'gcn_ln_b', 'proj_b', 'proj_ln_w', 'proj_ln_b',
]
PARAM_2D = ['mlp_w1', 'mlp_w2', 'gcn_w', 'proj_w']


def _host_consts():
    import ml_dtypes
    seg = np.minimum(np.arange(S) // L, K - 1)
    Lk = np.where(np.arange(K) < K - 1, 30.0, 45.0)
    mp = np.zeros((128, NT * K), np.float32)      # pooling mask, 1/L baked
    msc = np.zeros((K, NT * 128), np.float32)     # scatter mask, 0/1 (exact in bf16)
    for i in range(NT):
        for p in range(128):
            t = 128 * i + p
            if t < S:
                k = int(seg[t])
                mp[p, K * i + k] = 1.0 / Lk[k]
                msc[k, 128 * i + p] = 1.0
    ti = np.arange(K, dtype=np.float64)
    tatt = (np.exp(-GAMMA * np.abs(ti[:, None] - ti[None, :]))
            / np.sqrt(np.float64(H))).astype(np.float32)
    ident = np.eye(128, dtype=np.float32)
    onesrow = np.ones((1, 128), np.float32)
    oo2 = np.concatenate([np.full((128, 1), 1.0 / 128.0, np.float32),
                          np.full((128, 1), 1.0 / 256.0, np.float32)], 1)
    return {
        'c_mask_pool': mp,
        'c_mask_scat': msc.astype(ml_dtypes.bfloat16),
        'c_tattr': np.tile(tatt, (1, BL)).astype(np.float32),
        'c_eyefr': np.tile(np.eye(K, dtype=np.float32), (1, BL)),
        'c_ident': ident,
        'c_onesrow': onesrow,
        'c_oo2': oo2,
    }


def _build():
    import concourse.bass as bass
    import concourse.bacc as bacc
    import concourse.mybir as mybir
    import concourse.tile as tile

    f32 = mybir.dt.float32
    f32r = mybir.dt.float32r
    bf16 = mybir.dt.bfloat16
    AF = mybir.ActivationFunctionType
    OP = mybir.AluOpType
    AX = mybir.AxisListType
    MS = bass.MemorySpace

    def r32(t):
        return t[:, :].bitcast(f32r)

    nc = bacc.Bacc("TRN2", target_bir_lowering=False, debug=False)

    xs, outs = {}, {}
    for s in ('d', 'n', 'e'):
        xs[s] = nc.dram_tensor(f'x_{s}', [BL, S, D], f32, kind='ExternalInput')
        outs[s] = nc.dram_tensor(f'out_{s}', [BL, S, D], f32, kind='ExternalOutput')
    pr = {}
    for name in PARAM_1D:
        n_el = H if ('mlp' in name and 'w1' not in name) or 'gcn' in name else D
        pr[name] = nc.dram_tensor(name, [n_el, 1], f32, kind='ExternalInput')
    pr['mlp_w1'] = nc.dram_tensor('mlp_w1', [D, H], f32, kind='ExternalInput')
    pr['mlp_w2'] = nc.dram_tensor('mlp_w2', [H, H], f32, kind='ExternalInput')
    pr['gcn_w'] = nc.dram_tensor('gcn_w', [H, H], f32, kind='ExternalInput')
    pr['proj_w'] = nc.dram_tensor('proj_w', [H, D], f32, kind='ExternalInput')
    pr['c_mask_pool'] = nc.dram_tensor('c_mask_pool', [128, NT * K], f32, kind='ExternalInput')
    pr['c_mask_scat'] = nc.dram_tensor('c_mask_scat', [K, NT * 128], bf16, kind='ExternalInput')
    pr['c_tattr'] = nc.dram_tensor('c_tattr', [K, K * BL], f32, kind='ExternalInput')
    pr['c_eyefr'] = nc.dram_tensor('c_eyefr', [K, K * BL], f32, kind='ExternalInput')
    pr['c_ident'] = nc.dram_tensor('c_ident', [128, 128], f32, kind='ExternalInput')
    pr['c_onesrow'] = nc.dram_tensor('c_onesrow', [1, 128], f32, kind='ExternalInput')
    pr['c_oo2'] = nc.dram_tensor('c_oo2', [128, 2], f32, kind='ExternalInput')
    reg_out = nc.dram_tensor('reg_partial', [1, 1], f32, kind='ExternalOutput')

    W240 = BL * K  # 240 columns = 8 batches x 30 segments

    with tile.TileContext(nc) as tc, nc.allow_low_precision(reason="float32r is 32-bit storage"):
        import contextlib
        with contextlib.ExitStack() as ctx:
            cp = ctx.enter_context(tc.tile_pool(name='const', bufs=1))
            xp = ctx.enter_context(tc.tile_pool(name='xp', bufs=12))
            wp = ctx.enter_context(tc.tile_pool(name='work', bufs=2))
            pp = ctx.enter_context(tc.tile_pool(name='ps', bufs=1, space=MS.PSUM))

            def ct(shape, tag, dtype=f32):
                return cp.tile(shape, dtype, name=tag, tag=tag)

            # ---- constants / parameters into SBUF ----
            maskP = ct([128, NT * K], 'maskP')
            nc.sync.dma_start(maskP[:, :], pr['c_mask_pool'][:, :])
            maskS = ct([K, NT * 128], 'maskS', bf16)
            nc.sync.dma_start(maskS[:, :], pr['c_mask_scat'][:, :])
            tattr = ct([K, K * BL], 'tattr')
            nc.sync.dma_start(tattr[:, :], pr['c_tattr'][:, :])
            eyefr = ct([K, K * BL], 'eyefr')
            nc.sync.dma_start(eyefr[:, :], pr['c_eyefr'][:, :])
            ident = ct([128, 128], 'ident')
            nc.sync.dma_start(ident[:, :], pr['c_ident'][:, :])
            identb = ct([128, 128], 'identb', bf16)
            nc.scalar.activation(identb[:, :], ident[:, :], AF.Copy)
            one_row = ct([1, 128], 'one_row', f32r)
            nc.sync.dma_start(one_row[:, :], pr['c_onesrow'][:, :].bitcast(f32r))
            oo2 = ct([128, 2], 'oo2', f32r)
            nc.sync.dma_start(oo2[:, :], pr['c_oo2'][:, :].bitcast(f32r))

            w1a = ct([128, H], 'w1a', f32r)
            nc.sync.dma_start(w1a[:, :], r32(pr['mlp_w1'])[0:128, :])
            w1b = ct([128, H], 'w1b', f32r)
            nc.sync.dma_start(w1b[:, :], r32(pr['mlp_w1'])[128:256, :])
            w2 = ct([H, H], 'w2', f32r)
            nc.sync.dma_start(w2[:, :], pr['mlp_w2'][:, :].bitcast(f32r))
            gw = ct([H, H], 'gw', f32r)
            nc.sync.dma_start(gw[:, :], pr['gcn_w'][:, :].bitcast(f32r))
            pw = ct([H, D], 'pw', f32r)
            nc.sync.dma_start(pw[:, :], pr['proj_w'][:, :].bitcast(f32r))

            def col(name, lo, n, tag):
                t = ct([n, 1], tag)
                nc.sync.dma_start(t[:, :], pr[name][lo:lo + n, :])
                return t

            enc_ln = {}
            for s in ('d', 'n', 'e'):
                enc_ln[s] = (
                    col(f'ln_{s}_w', 0, 128, f'lw0{s}'),
                    col(f'ln_{s}_w', 128, 128, f'lw1{s}'),
                    col(f'ln_{s}_b', 0, 128, f'lb0{s}'),
                    col(f'ln_{s}_b', 128, 128, f'lb1{s}'),
                )
            b1c = col('mlp_b1', 0, H, 'b1c')
            l1w = col('mlp_ln1_w', 0, H, 'l1w')
            l1b = col('mlp_ln1_b', 0, H, 'l1b')
            b2c = col('mlp_b2', 0, H, 'b2c')
            l2w = col('mlp_ln2_w', 0, H, 'l2w')
            l2b = col('mlp_ln2_b', 0, H, 'l2b')
            gcw = col('gcn_ln_w', 0, H, 'gcw')
            gcb = col('gcn_ln_b', 0, H, 'gcb')
            pb0 = col('proj_b', 0, 128, 'pb0')
            pb1 = col('proj_b', 128, 128, 'pb1')
            plw0 = col('proj_ln_w', 0, 128, 'plw0')
            plw1 = col('proj_ln_w', 128, 128, 'plw1')
            plb0 = col('proj_ln_b', 0, 128, 'plb0')
            plb1 = col('proj_ln_b', 128, 128, 'plb1')

            ones_col = ct([128, 1], 'ones_col')
            nc.vector.memset(ones_col[:, :], 1.0)

            gcnT = {}  # stream -> [128, 240] final GCN output (u^T), kept for reg

            def cols30(b):
                return slice(K * b, K * b + K)

            # ============ layer-norm helper (features on partitions) ============
            # stats matmul lhsT is ones/denom so psum holds [mu | E[x^2]] rows.
            def ln_feat(tag, val_ps_list, bias_cols, lnw_list, lnb_list, relu, denom,
                        out_tags, out_bufs=2, out_dtype=None):
                nh = len(val_ps_list)
                oo = oo2[:, 0:1] if denom == 128 else oo2[:, 1:2]
                hv = []
                for h in range(nh):
                    hvt = wp.tile([128, 2 * W240], f32r, name=f'{tag}hv{h}', tag=f'hv{h}',
                                  bufs=2)
                    if bias_cols[h] is None:
                        nc.scalar.activation(hvt[:, 0:W240], val_ps_list[h], AF.Copy)
                        nc.scalar.activation(hvt[:, W240:2 * W240], val_ps_list[h],
                                             AF.Square)
                    else:
                        nc.scalar.activation(hvt[:, 0:W240], val_ps_list[h], AF.Identity,
                                             bias=bias_cols[h][:, 0:1], scale=1.0)
                        nc.scalar.activation(hvt[:, W240:2 * W240], val_ps_list[h],
                                             AF.Square, bias=bias_cols[h][:, 0:1],
                                             scale=1.0)
                    hv.append(hvt)
                st_ps = pp.tile([1, 2 * W240], f32, name=f'{tag}st', tag='sm', bufs=3)
                for h in range(nh):
                    nc.tensor.matmul(st_ps[:, :], oo, hv[h][:, :],
                                     start=(h == 0), stop=(h == nh - 1))
                # rows: mu = st[0:W], ex2 = st[W:2W] (both already / denom)
                musq = wp.tile([1, W240], f32, name=f'{tag}musq', tag='row', bufs=6)
                nc.scalar.activation(musq[:, :], st_ps[0:1, 0:W240], AF.Square)
                nc.vector.tensor_scalar(musq[:, :], musq[:, :], EPS, None, op0=OP.subtract)
                ve = wp.tile([1, W240], f32, name=f'{tag}ve', tag='row', bufs=6)
                nc.vector.tensor_tensor(ve[:, :], st_ps[0:1, W240:2 * W240], musq[:, :],
                                        op=OP.subtract)
                nc.vector.reciprocal(ve[:, :], ve[:, :])
                rrow = wp.tile([1, W240], f32r, name=f'{tag}rrow', tag='row', bufs=6)
                nc.scalar.activation(rrow[:, :], ve[:, :], AF.Sqrt)
                murrow = wp.tile([1, W240], f32r, name=f'{tag}murrow', tag='row', bufs=6)
                nc.vector.tensor_tensor(murrow[:, :], st_ps[0:1, 0:W240], rrow[:, :],
                                        op=OP.mult)
                rb_ps = pp.tile([128, W240], f32, name=f'{tag}rb', tag='sm', bufs=3)
                nc.tensor.matmul(rb_ps[:, :], one_row[:, :], rrow[:, :])
                murb_ps = pp.tile([128, W240], f32, name=f'{tag}murb', tag='sm', bufs=3)
                nc.tensor.matmul(murb_ps[:, :], one_row[:, :], murrow[:, :])
                outs_sb = []
                for h in range(nh):
                    xn = hv[h][:, W240:2 * W240]
                    nc.vector.tensor_tensor(xn, hv[h][:, 0:W240], rb_ps[:, :], op=OP.mult)
                    nc.vector.tensor_tensor(xn, xn, murb_ps[:, :], op=OP.subtract)
                    o = wp.tile([128, W240], out_dtype or f32, name=f'{tag}o{h}',
                                tag=out_tags[h], bufs=out_bufs)
                    nc.scalar.activation(o[:, :], xn, AF.Relu if relu else AF.Identity,
                                         bias=lnb_list[h][:, 0:1], scale=lnw_list[h][:, 0:1])
                    outs_sb.append(o)
                return outs_sb

            # ==================== per-stream pipeline ====================
            blend_flip = 0
            for s in ('d', 'n', 'e'):
                x_dram = xs[s]
                out_dram = outs[s]
                lw0, lw1, lb0, lb1 = enc_ln[s]

                segT0 = wp.tile([128, W240], f32r, name=f'segT0{s}', tag='segT0')
                segT1 = wp.tile([128, W240], f32r, name=f'segT1{s}', tag='segT1')

                xbs = []
                xb16s = []
                # -------- P1: load, token stats, bf16 copy, pooling, transpose --------
                # c_k = (1/256) * sum_d pool[k,d] exactly equals the mean
                # correction, so no per-token mean column is needed.
                for b in range(BL):
                    xb = xp.tile([128, NT, 256], f32r, name=f'xb{s}{b}', tag='xb', bufs=10)
                    xbs.append(xb)
                    xb16 = xp.tile([128, NT * 256], bf16, name=f'xb16{s}{b}', tag='xb16',
                                   bufs=10)
                    xb16s.append(xb16)
                    st2 = wp.tile([128, 2, NT], f32, name=f'st2{s}{b}', tag='st2', bufs=3)
                    nc.vector.memset(st2[:, :, NT - 1], 1.0)
                    for i in range(NT):
                        pc = 128 if i < NT - 1 else S - 128 * (NT - 1)
                        xv = xb[0:pc, i, :]
                        nc.sync.dma_start(xv, x_dram[b, 128 * i:128 * i + pc, :].bitcast(f32r))
                        # bf16 copy fused with token sum (ACT/DVE alternating)
                        xo = xb16[0:pc, 256 * i:256 * i + 256]
                        nc.vector.tensor_scalar(xo, xv, 1.0, 0.0, op0=OP.mult,
                                                op1=OP.add,
                                                accum_out=st2[0:pc, 0, i:i + 1])
                        # sum of squares from the bf16 copy (split DVE/ACT)
                        sqd = wp.tile([128, 256], bf16, name=f'sqd{s}{b}{i}', tag='sqd',
                                      bufs=4)
                        if i % 3 == 0:
                            nc.vector.scalar_tensor_tensor(sqd[0:pc, :], xo, 1.0, xo,
                                                           op0=OP.mult, op1=OP.mult,
                                                           accum_out=st2[0:pc, 1, i:i + 1])
                        else:
                            nc.scalar.activation(sqd[0:pc, :], xo, AF.Square,
                                                 accum_out=st2[0:pc, 1, i:i + 1])
                    mu = wp.tile([128, NT], f32, name=f'mu{s}{b}', tag='mu', bufs=3)
                    nc.scalar.activation(mu[:, :], st2[:, 0, :], AF.Copy, scale=1.0 / 256.0)
                    musq = wp.tile([128, NT], f32, name=f'musq{s}{b}', tag='musq', bufs=3)
                    nc.scalar.activation(musq[:, :], mu[:, :], AF.Square)
                    nc.vector.tensor_scalar(musq[:, :], musq[:, :], EPS, None,
                                            op0=OP.subtract)
                    ve = wp.tile([128, NT], f32, name=f've{s}{b}', tag='vep', bufs=3)
                    nc.vector.scalar_tensor_tensor(ve[:, :], st2[:, 1, :], 1.0 / 256.0,
                                                   musq[:, :], op0=OP.mult, op1=OP.subtract)
                    nc.vector.reciprocal(ve[:, :], ve[:, :])
                    rt = wp.tile([128, NT], f32, name=f'rt{s}{b}', tag='rt', bufs=3)
                    nc.scalar.activation(rt[:, :], ve[:, :], AF.Sqrt)
                    wall = wp.tile([128, NT, K], bf16, name=f'wall{s}{b}', tag='wall',
                                   bufs=4)
                    nc.vector.tensor_tensor(
                        wall[:, :, :],
                        maskP[:, :].rearrange("p (n k) -> p n k", k=K),
                        rt[:, :].broadcast_to([128, NT, K]),
                        op=OP.mult)
                    seg_ps = pp.tile([K, 256], f32, name=f'segps{s}{b}', tag='sm', bufs=3)
                    for i in range(NT):
                        pc = 128 if i < NT - 1 else S - 128 * (NT - 1)
                        nc.tensor.matmul(seg_ps[:, :], wall[0:pc, i, :],
                                         xb16[0:pc, 256 * i:256 * i + 256],
                                         start=(i == 0), stop=(i == NT - 1))
                    srow = wp.tile([K, 2], f32, name=f'srow{s}{b}', tag='srow', bufs=3)
                    nc.vector.reduce_sum(srow[:, 0:1], seg_ps[:, :], axis=AX.X)
                    nc.vector.tensor_scalar(srow[:, 1:2], srow[:, 0:1], -1.0 / 256.0,
                                            None, op0=OP.mult)
                    segc = wp.tile([K, 256], bf16, name=f'segc{s}{b}', tag='segc', bufs=3)
                    nc.scalar.activation(segc[:, :], seg_ps[:, :], AF.Identity,
                                         bias=srow[:, 1:2], scale=1.0)
                    tp0 = pp.tile([128, K], bf16, name=f'tp0{s}{b}', tag='smb', bufs=2)
                    nc.tensor.transpose(tp0[:, :], segc[:, 0:128], identb[0:K, 0:K])
                    nc.scalar.activation(segT0[:, cols30(b)], tp0[:, :], AF.Identity,
                                         bias=lb0[:, 0:1], scale=lw0[:, 0:1])
                    tp1 = pp.tile([128, K], bf16, name=f'tp1{s}{b}', tag='smb', bufs=2)
                    nc.tensor.transpose(tp1[:, :], segc[:, 128:256], identb[0:K, 0:K])
                    nc.scalar.activation(segT1[:, cols30(b)], tp1[:, :], AF.Identity,
                                         bias=lb1[:, 0:1], scale=lw1[:, 0:1])

                # -------- P3: MLP --------
                h1_ps = pp.tile([128, W240], f32, name=f'h1ps{s}', tag='big', bufs=3)
                nc.tensor.matmul(h1_ps[:, :], w1a[:, :], segT0[:, :],
                                 start=True, stop=False)
                nc.tensor.matmul(h1_ps[:, :], w1b[:, :], segT1[:, :],
                                 start=False, stop=True)
                (h1_sb,) = ln_feat(f'l1{s}', [h1_ps[:, :]], [b1c], [l1w], [l1b],
                                   relu=True, denom=H, out_tags=['lnA'], out_dtype=f32r)

                h2_ps = pp.tile([128, W240], f32, name=f'h2ps{s}', tag='big', bufs=3)
                nc.tensor.matmul(h2_ps[:, :], w2[:, :], h1_sb[:, :])
                (encT,) = ln_feat(f'l2{s}', [h2_ps[:, :]], [b2c], [l2w], [l2b],
                                  relu=False, denom=H, out_tags=['lnB'], out_dtype=f32r)

                # -------- P4: GCN --------
                msg_ps = pp.tile([128, W240], f32, name=f'msgps{s}', tag='big', bufs=3)
                nc.tensor.matmul(msg_ps[:, :], gw[:, :], encT[:, :])
                msgT = wp.tile([128, W240], bf16, name=f'msgT{s}', tag='msgT')
                nc.scalar.activation(msgT[:, :], msg_ps[:, :], AF.Copy, scale=0.5)

                updT_ps = pp.tile([128, W240], f32, name=f'updps{s}', tag='big', bufs=3)
                sim_ps = pp.tile([K, W240], f32, name=f'sim{s}', tag='big', bufs=3)
                for b in range(BL):
                    nc.tensor.matmul(sim_ps[:, cols30(b)], encT[:, cols30(b)],
                                     encT[:, cols30(b)])
                z = wp.tile([K, W240], f32, name=f'z{s}', tag='z', bufs=2)
                nc.vector.tensor_tensor(z[:, :], sim_ps[:, :], tattr[:, :], op=OP.mult)
                zneg = wp.tile([K, BL], f32, name=f'zneg{s}', tag='zneg', bufs=2)
                nc.vector.reduce_max(zneg[:, :], z[:, :].rearrange("p (n k) -> p n k", k=K),
                                     axis=AX.X, negate=True)
                es = wp.tile([K, BL], f32, name=f'es{s}', tag='es', bufs=2)
                e_sb = wp.tile([K, W240], f32, name=f'e{s}', tag='esb', bufs=2)
                for b in range(BL):
                    nc.scalar.activation(e_sb[:, cols30(b)], z[:, cols30(b)], AF.Exp,
                                         bias=zneg[:, b:b + 1], scale=1.0,
                                         accum_out=es[:, b:b + 1])
                nc.vector.reciprocal(es[:, :], es[:, :])
                sm2 = wp.tile([K, W240], bf16, name=f'sm2{s}', tag='sm2', bufs=2)
                for b in range(BL):
                    nc.scalar.activation(sm2[:, cols30(b)], e_sb[:, cols30(b)], AF.Copy,
                                         scale=es[:, b:b + 1])
                tr_ps = pp.tile([K, W240], bf16, name=f'tr{s}', tag='big', bufs=3)
                for b in range(BL):
                    nc.tensor.transpose(tr_ps[:, cols30(b)], sm2[:, cols30(b)],
                                        identb[0:K, 0:K])
                at = wp.tile([K, W240], bf16, name=f'at{s}', tag='at', bufs=2)
                nc.vector.tensor_tensor(at[:, :], tr_ps[:, :], eyefr[:, :], op=OP.add)
                for b in range(BL):
                    trm_ps = pp.tile([K, 128], bf16, name=f'trm{s}{b}', tag='smb', bufs=2)
                    nc.tensor.transpose(trm_ps[:, :], msgT[:, cols30(b)], identb[:, :])
                    msg_sb = wp.tile([K, 128], bf16, name=f'msgsb{s}{b}', tag='msgsb', bufs=3)
                    nc.scalar.activation(msg_sb[:, :], trm_ps[:, :], AF.Copy)
                    nc.tensor.matmul(updT_ps[:, cols30(b)], msg_sb[:, :],
                                     at[:, cols30(b)])

                (gc,) = ln_feat(f'gc{s}', [updT_ps[:, :]], [None], [gcw], [gcb],
                                relu=True, denom=H, out_tags=[f'gcnT{s}'], out_bufs=1,
                                out_dtype=f32r)
                gcnT[s] = gc

                # -------- P5: projection --------
                pj0_ps = pp.tile([128, W240], f32, name=f'pj0{s}', tag='big', bufs=3)
                nc.tensor.matmul(pj0_ps[:, :], pw[:, 0:128], gc[:, :])
                pj1_ps = pp.tile([128, W240], f32, name=f'pj1{s}', tag='big', bufs=3)
                nc.tensor.matmul(pj1_ps[:, :], pw[:, 128:256], gc[:, :])
                pT0, pT1 = ln_feat(f'pj{s}', [pj0_ps[:, :], pj1_ps[:, :]], [pb0, pb1],
                                   [plw0, plw1], [plb0, plb1], relu=False, denom=D,
                                   out_tags=['pT0', 'pT1'], out_dtype=bf16)

                # -------- P6: scatter + blend per batch --------
                for b in range(BL):
                    proj_sb = wp.tile([K, D], bf16, name=f'projsb{s}{b}', tag='projsb',
                                      bufs=4)
                    tq0 = pp.tile([K, 128], bf16, name=f'tq0{s}{b}', tag='smb', bufs=2)
                    nc.tensor.transpose(tq0[:, :], pT0[:, cols30(b)], identb[:, :])
                    nc.scalar.activation(proj_sb[:, 0:128], tq0[:, :], AF.Copy,
                                         scale=1.0 - ALPHA)
                    tq1 = pp.tile([K, 128], bf16, name=f'tq1{s}{b}', tag='smb', bufs=2)
                    nc.tensor.transpose(tq1[:, :], pT1[:, cols30(b)], identb[:, :])
                    nc.scalar.activation(proj_sb[:, 128:256], tq1[:, :], AF.Copy,
                                         scale=1.0 - ALPHA)
                    xb = xbs[b]
                    for i in range(NT):
                        pc = 128 if i < NT - 1 else S - 128 * (NT - 1)
                        g_ps = pp.tile([128, D], f32, name=f'g{s}{b}{i}', tag='big', bufs=3)
                        nc.tensor.matmul(g_ps[0:pc, :],
                                         maskS[:, 128 * i:128 * i + pc],
                                         proj_sb[:, :])
                        o_sb = wp.tile([128, D], f32, name=f'osb{s}{b}{i}', tag='osb',
                                       bufs=6)
                        nc.vector.scalar_tensor_tensor(o_sb[0:pc, :],
                                                       xb[0:pc, i, :], ALPHA,
                                                       g_ps[0:pc, :],
                                                       op0=OP.mult, op1=OP.add)
                        nc.gpsimd.dma_start(out_dram[b, 128 * i:128 * i + pc, :],
                                            o_sb[0:pc, :])

            # ==================== reg loss partial ====================
            pairs = [('d', 'd'), ('n', 'n'), ('e', 'e'), ('d', 'n'), ('d', 'e'), ('n', 'e')]
            acc = wp.tile([128, 6], f32, name='regacc', tag='regacc', bufs=1)
            dump = wp.tile([128, W240], f32, name='regdump', tag='regdump', bufs=2)
            for j, (a, bb) in enumerate(pairs):
                nc.vector.tensor_tensor(dump[:, :], gcnT[a][:, :], gcnT[bb][:, :],
                                        op=OP.mult)
                nc.vector.reduce_sum(acc[:, j:j + 1], dump[:, :], axis=AX.X)
            tsum = wp.tile([128, 1], f32, name='tsum', tag='tsum', bufs=1)
            nc.vector.tensor_tensor(tsum[:, :], acc[:, 0:1], acc[:, 1:2], op=OP.add)
            nc.vector.tensor_tensor(tsum[:, :], tsum[:, :], acc[:, 2:3], op=OP.add)
            usum = wp.tile([128, 1], f32, name='usum', tag='usum', bufs=1)
            nc.vector.tensor_tensor(usum[:, :], acc[:, 3:4], acc[:, 4:5], op=OP.add)
            nc.vector.tensor_tensor(usum[:, :], usum[:, :], acc[:, 5:6], op=OP.add)
            wv = wp.tile([128, 1], f32, name='wv', tag='wv', bufs=1)
            nc.vector.scalar_tensor_tensor(wv[:, :], usum[:, :], -LAM, tsum[:, :],
                                           op0=OP.mult, op1=OP.add)
            rg_ps = pp.tile([1, 1], f32, name='rgps', tag='sm', bufs=3)
            nc.tensor.matmul(rg_ps[:, :], wv[:, :], ones_col[:, :])
            rg_sb = wp.tile([1, 1], f32, name='rgsb', tag='rgsb', bufs=1)
            nc.scalar.activation(rg_sb[:, :], rg_ps[:, :], AF.Copy)
            nc.sync.dma_start(reg_out[:, :], rg_sb[:, :])

    nc.compile()
    return nc


_PROG = None


def _get_prog():
    global _PROG
    if _PROG is None:
        _PROG = _build()
    return _PROG


def _make_in_maps(inputs):
    consts = _host_consts()
    base = {}
    for name in PARAM_1D:
        base[name] = np.ascontiguousarray(
            np.asarray(inputs[name], dtype=np.float32).reshape(-1, 1))
    for name in PARAM_2D:
        base[name] = np.ascontiguousarray(np.asarray(inputs[name], dtype=np.float32))
    base.update(consts)
    in_maps = []
    for c in range(NCORES):
        m = dict(base)
        for s in ('d', 'n', 'e'):
            m[f'x_{s}'] = np.ascontiguousarray(
                np.asarray(inputs[f'x_{s}'], dtype=np.float32)[c * BL:(c + 1) * BL])
        in_maps.append(m)
    return in_maps


def _run(in_maps, trace=False):
    from concourse.bass_utils import run_bass_kernel_spmd
    nc = _get_prog()
    return run_bass_kernel_spmd(nc, in_maps, core_ids=list(range(NCORES)), trace=trace)


def kernel(**inputs):
    in_maps = _make_in_maps(inputs)
    res = _run(in_maps, trace=False)
    outs = res.results
    out_d = np.concatenate([outs[c]['out_d'] for c in range(NCORES)], axis=0)
    out_n = np.concatenate([outs[c]['out_n'] for c in range(NCORES)], axis=0)
    out_e = np.concatenate([outs[c]['out_e'] for c in range(NCORES)], axis=0)
    reg = np.float32(sum(float(outs[c]['reg_partial'][0, 0]) for c in range(NCORES)) / B)
    return out_d, out_n, out_e, reg


# revision 30
# speedup vs baseline: 1.0417x; 1.0417x over previous
"""Trainium2 Bass kernel for DualConsistencyRegularization.

Contract: kernel(**inputs) takes FULL unsharded inputs (B=64) and returns
(out_d, out_n, out_e, reg_loss) matching reference.reference(**inputs).

Strategy: pure data parallel over batch (8 per core x 8 cores). Each core
runs an identical NEFF on its batch shard; reg_loss partials are summed on
host (mean over batch == sum of per-core partial sums / B).

Kernel structure per core, per stream (d/n/e):
 - P1 token LN + segment pooling: per 128-token tile, token mean (DVE
   reduce) and sum-of-squares (ACT Square accum) feed rsqrt(var+eps); the
   pooling matmul contracts tokens with the per-token r baked into a mask
   (1/L also baked), with the token means as rhs column 256 so the mean
   correction comes out as psum column 256. float32r keeps PE at full rate.
 - MLP/GCN/proj run feature-on-partitions at [128, 240] (8 batches x 30
   segments); LN over the partition dim uses ones-matmul stats + K=1
   broadcast matmuls. Softmax row sums make A.sum == 2 exactly, so
   normalization is (softmax + I)/2, with the 1/2 folded into the msg copy.
 - Scatter+blend: one-hot mask matmul gathers proj rows per token tile
   ((1-alpha) baked into the mask); blend alternates between a fused DVE
   scalar_tensor_tensor and a PE alpha*I accumulate + ACT copy to balance
   engines.
"""

import numpy as np

B, S, D = 64, 915, 256
K, H = 30, 128
L = 30
BASE = 870
GAMMA, LAM, ALPHA = 0.1, 0.5, 0.9
EPS = 1e-5
NCORES = 8
BL = B // NCORES  # 8 batches per core
NT = 8            # token tiles per batch: 7 full x128 + 1 x19

PARAM_1D = [
    'ln_d_w', 'ln_d_b', 'ln_n_w', 'ln_n_b', 'ln_e_w', 'ln_e_b',
    'mlp_b1', 'mlp_ln1_w', 'mlp_ln1_b', 'mlp_b2', 'mlp_ln2_w', 'mlp_ln2_b',
    'gcn_ln_w', 'gcn_ln_b', 'proj_b', 'proj_ln_w', 'proj_ln_b',
]
PARAM_2D = ['mlp_w1', 'mlp_w2', 'gcn_w', 'proj_w']


def _host_consts():
    import ml_dtypes
    seg = np.minimum(np.arange(S) // L, K - 1)
    Lk = np.where(np.arange(K) < K - 1, 30.0, 45.0)
    mp = np.zeros((128, NT * K), np.float32)      # pooling mask, 1/L baked
    msc = np.zeros((K, NT * 128), np.float32)     # scatter mask, 0/1 (exact in bf16)
    for i in range(NT):
        for p in range(128):
            t = 128 * i + p
            if t < S:
                k = int(seg[t])
                mp[p, K * i + k] = 1.0 / Lk[k]
                msc[k, 128 * i + p] = 1.0
    ti = np.arange(K, dtype=np.float64)
    tatt = (np.exp(-GAMMA * np.abs(ti[:, None] - ti[None, :]))
            / np.sqrt(np.float64(H))).astype(np.float32)
    ident = np.eye(128, dtype=np.float32)
    onesrow = np.ones((1, 128), np.float32)
    oo2 = np.concatenate([np.full((128, 1), 1.0 / 128.0, np.float32),
                          np.full((128, 1), 1.0 / 256.0, np.float32)], 1)
    return {
        'c_mask_pool': mp,
        'c_mask_scat': msc.astype(ml_dtypes.bfloat16),
        'c_tattr': np.tile(tatt, (1, BL)).astype(np.float32),
        'c_eyefr': np.tile(np.eye(K, dtype=np.float32), (1, BL)),
        'c_ident': ident,
        'c_onesrow': onesrow,
        'c_oo2': oo2,
    }


def _build():
    import concourse.bass as bass
    import concourse.bacc as bacc
    import concourse.mybir as mybir
    import concourse.tile as tile

    f32 = mybir.dt.float32
    f32r = mybir.dt.float32r
    bf16 = mybir.dt.bfloat16
    AF = mybir.ActivationFunctionType
    OP = mybir.AluOpType
    AX = mybir.AxisListType
    MS = bass.MemorySpace

    def r32(t):
        return t[:, :].bitcast(f32r)

    nc = bacc.Bacc("TRN2", target_bir_lowering=False, debug=False)

    xs, outs = {}, {}
    for s in ('d', 'n', 'e'):
        xs[s] = nc.dram_tensor(f'x_{s}', [BL, S, D], f32, kind='ExternalInput')
        outs[s] = nc.dram_tensor(f'out_{s}', [BL, S, D], f32, kind='ExternalOutput')
    pr = {}
    for name in PARAM_1D:
        n_el = H if ('mlp' in name and 'w1' not in name) or 'gcn' in name else D
        pr[name] = nc.dram_tensor(name, [n_el, 1], f32, kind='ExternalInput')
    pr['mlp_w1'] = nc.dram_tensor('mlp_w1', [D, H], f32, kind='ExternalInput')
    pr['mlp_w2'] = nc.dram_tensor('mlp_w2', [H, H], f32, kind='ExternalInput')
    pr['gcn_w'] = nc.dram_tensor('gcn_w', [H, H], f32, kind='ExternalInput')
    pr['proj_w'] = nc.dram_tensor('proj_w', [H, D], f32, kind='ExternalInput')
    pr['c_mask_pool'] = nc.dram_tensor('c_mask_pool', [128, NT * K], f32, kind='ExternalInput')
    pr['c_mask_scat'] = nc.dram_tensor('c_mask_scat', [K, NT * 128], bf16, kind='ExternalInput')
    pr['c_tattr'] = nc.dram_tensor('c_tattr', [K, K * BL], f32, kind='ExternalInput')
    pr['c_eyefr'] = nc.dram_tensor('c_eyefr', [K, K * BL], f32, kind='ExternalInput')
    pr['c_ident'] = nc.dram_tensor('c_ident', [128, 128], f32, kind='ExternalInput')
    pr['c_onesrow'] = nc.dram_tensor('c_onesrow', [1, 128], f32, kind='ExternalInput')
    pr['c_oo2'] = nc.dram_tensor('c_oo2', [128, 2], f32, kind='ExternalInput')
    reg_out = nc.dram_tensor('reg_partial', [1, 1], f32, kind='ExternalOutput')

    W240 = BL * K  # 240 columns = 8 batches x 30 segments

    with tile.TileContext(nc) as tc, nc.allow_low_precision(reason="float32r is 32-bit storage"):
        import contextlib
        with contextlib.ExitStack() as ctx:
            cp = ctx.enter_context(tc.tile_pool(name='const', bufs=1))
            xp = ctx.enter_context(tc.tile_pool(name='xp', bufs=12))
            wp = ctx.enter_context(tc.tile_pool(name='work', bufs=2))
            pp = ctx.enter_context(tc.tile_pool(name='ps', bufs=1, space=MS.PSUM))

            def ct(shape, tag, dtype=f32):
                return cp.tile(shape, dtype, name=tag, tag=tag)

            # ---- constants / parameters into SBUF ----
            maskP = ct([128, NT * K], 'maskP')
            nc.sync.dma_start(maskP[:, :], pr['c_mask_pool'][:, :])
            maskS = ct([K, NT * 128], 'maskS', bf16)
            nc.sync.dma_start(maskS[:, :], pr['c_mask_scat'][:, :])
            tattr = ct([K, K * BL], 'tattr')
            nc.sync.dma_start(tattr[:, :], pr['c_tattr'][:, :])
            eyefr = ct([K, K * BL], 'eyefr')
            nc.sync.dma_start(eyefr[:, :], pr['c_eyefr'][:, :])
            ident = ct([128, 128], 'ident')
            nc.sync.dma_start(ident[:, :], pr['c_ident'][:, :])
            identb = ct([128, 128], 'identb', bf16)
            nc.scalar.activation(identb[:, :], ident[:, :], AF.Copy)
            one_row = ct([1, 128], 'one_row', f32r)
            nc.sync.dma_start(one_row[:, :], pr['c_onesrow'][:, :].bitcast(f32r))
            oo2 = ct([128, 2], 'oo2', f32r)
            nc.sync.dma_start(oo2[:, :], pr['c_oo2'][:, :].bitcast(f32r))

            w1a = ct([128, H], 'w1a', f32r)
            nc.sync.dma_start(w1a[:, :], r32(pr['mlp_w1'])[0:128, :])
            w1b = ct([128, H], 'w1b', f32r)
            nc.sync.dma_start(w1b[:, :], r32(pr['mlp_w1'])[128:256, :])
            w2 = ct([H, H], 'w2', f32r)
            nc.sync.dma_start(w2[:, :], pr['mlp_w2'][:, :].bitcast(f32r))
            gw = ct([H, H], 'gw', f32r)
            nc.sync.dma_start(gw[:, :], pr['gcn_w'][:, :].bitcast(f32r))
            pw = ct([H, D], 'pw', f32r)
            nc.sync.dma_start(pw[:, :], pr['proj_w'][:, :].bitcast(f32r))

            def col(name, lo, n, tag):
                t = ct([n, 1], tag)
                nc.sync.dma_start(t[:, :], pr[name][lo:lo + n, :])
                return t

            enc_ln = {}
            for s in ('d', 'n', 'e'):
                enc_ln[s] = (
                    col(f'ln_{s}_w', 0, 128, f'lw0{s}'),
                    col(f'ln_{s}_w', 128, 128, f'lw1{s}'),
                    col(f'ln_{s}_b', 0, 128, f'lb0{s}'),
                    col(f'ln_{s}_b', 128, 128, f'lb1{s}'),
                )
            b1c = col('mlp_b1', 0, H, 'b1c')
            l1w = col('mlp_ln1_w', 0, H, 'l1w')
            l1b = col('mlp_ln1_b', 0, H, 'l1b')
            b2c = col('mlp_b2', 0, H, 'b2c')
            l2w = col('mlp_ln2_w', 0, H, 'l2w')
            l2b = col('mlp_ln2_b', 0, H, 'l2b')
            gcw = col('gcn_ln_w', 0, H, 'gcw')
            gcb = col('gcn_ln_b', 0, H, 'gcb')
            pb0 = col('proj_b', 0, 128, 'pb0')
            pb1 = col('proj_b', 128, 128, 'pb1')
            plw0 = col('proj_ln_w', 0, 128, 'plw0')
            plw1 = col('proj_ln_w', 128, 128, 'plw1')
            plb0 = col('proj_ln_b', 0, 128, 'plb0')
            plb1 = col('proj_ln_b', 128, 128, 'plb1')

            ones_col = ct([128, 1], 'ones_col')
            nc.vector.memset(ones_col[:, :], 1.0)

            gcnT = {}  # stream -> [128, 240] final GCN output (u^T), kept for reg

            def cols30(b):
                return slice(K * b, K * b + K)

            # ============ layer-norm helper (features on partitions) ============
            # stats matmul lhsT is ones/denom so psum holds [mu | E[x^2]] rows.
            def ln_feat(tag, val_ps_list, bias_cols, lnw_list, lnb_list, relu, denom,
                        out_tags, out_bufs=2, out_dtype=None):
                nh = len(val_ps_list)
                oo = oo2[:, 0:1] if denom == 128 else oo2[:, 1:2]
                hv = []
                for h in range(nh):
                    hvt = wp.tile([128, 2 * W240], f32r, name=f'{tag}hv{h}', tag=f'hv{h}',
                                  bufs=2)
                    if bias_cols[h] is None:
                        nc.scalar.activation(hvt[:, 0:W240], val_ps_list[h], AF.Copy)
                        nc.scalar.activation(hvt[:, W240:2 * W240], val_ps_list[h],
                                             AF.Square)
                    else:
                        nc.scalar.activation(hvt[:, 0:W240], val_ps_list[h], AF.Identity,
                                             bias=bias_cols[h][:, 0:1], scale=1.0)
                        nc.scalar.activation(hvt[:, W240:2 * W240], val_ps_list[h],
                                             AF.Square, bias=bias_cols[h][:, 0:1],
                                             scale=1.0)
                    hv.append(hvt)
                st_ps = pp.tile([1, 2 * W240], f32, name=f'{tag}st', tag='sm', bufs=3)
                for h in range(nh):
                    nc.tensor.matmul(st_ps[:, :], oo, hv[h][:, :],
                                     start=(h == 0), stop=(h == nh - 1))
                # rows: mu = st[0:W], ex2 = st[W:2W] (both already / denom)
                musq = wp.tile([1, W240], f32, name=f'{tag}musq', tag='row', bufs=6)
                nc.scalar.activation(musq[:, :], st_ps[0:1, 0:W240], AF.Square)
                nc.vector.tensor_scalar(musq[:, :], musq[:, :], EPS, None, op0=OP.subtract)
                ve = wp.tile([1, W240], f32, name=f'{tag}ve', tag='row', bufs=6)
                nc.vector.tensor_tensor(ve[:, :], st_ps[0:1, W240:2 * W240], musq[:, :],
                                        op=OP.subtract)
                nc.vector.reciprocal(ve[:, :], ve[:, :])
                rrow = wp.tile([1, W240], f32r, name=f'{tag}rrow', tag='row', bufs=6)
                nc.scalar.activation(rrow[:, :], ve[:, :], AF.Sqrt)
                murrow = wp.tile([1, W240], f32r, name=f'{tag}murrow', tag='row', bufs=6)
                nc.vector.tensor_tensor(murrow[:, :], st_ps[0:1, 0:W240], rrow[:, :],
                                        op=OP.mult)
                rb_ps = pp.tile([128, W240], f32, name=f'{tag}rb', tag='sm', bufs=3)
                nc.tensor.matmul(rb_ps[:, :], one_row[:, :], rrow[:, :])
                murb_ps = pp.tile([128, W240], f32, name=f'{tag}murb', tag='sm', bufs=3)
                nc.tensor.matmul(murb_ps[:, :], one_row[:, :], murrow[:, :])
                outs_sb = []
                for h in range(nh):
                    xn = hv[h][:, W240:2 * W240]
                    nc.vector.tensor_tensor(xn, hv[h][:, 0:W240], rb_ps[:, :], op=OP.mult)
                    nc.vector.tensor_tensor(xn, xn, murb_ps[:, :], op=OP.subtract)
                    o = wp.tile([128, W240], out_dtype or f32, name=f'{tag}o{h}',
                                tag=out_tags[h], bufs=out_bufs)
                    nc.scalar.activation(o[:, :], xn, AF.Relu if relu else AF.Identity,
                                         bias=lnb_list[h][:, 0:1], scale=lnw_list[h][:, 0:1])
                    outs_sb.append(o)
                return outs_sb

            # ==================== per-stream pipeline ====================
            blend_flip = 0
            for s in ('d', 'n', 'e'):
                x_dram = xs[s]
                out_dram = outs[s]
                lw0, lw1, lb0, lb1 = enc_ln[s]

                segT0 = wp.tile([128, W240], f32r, name=f'segT0{s}', tag='segT0')
                segT1 = wp.tile([128, W240], f32r, name=f'segT1{s}', tag='segT1')

                xbs = []
                xb16s = []
                # -------- P1: load, token stats, bf16 copy, pooling, transpose --------
                # c_k = (1/256) * sum_d pool[k,d] exactly equals the mean
                # correction, so no per-token mean column is needed.
                for b in range(BL):
                    xb = xp.tile([128, NT, 256], f32r, name=f'xb{s}{b}', tag='xb', bufs=12)
                    xbs.append(xb)
                    xb16 = xp.tile([128, NT * 256], bf16, name=f'xb16{s}{b}', tag='xb16',
                                   bufs=12)
                    xb16s.append(xb16)
                    st2 = wp.tile([128, 2, NT], f32, name=f'st2{s}{b}', tag='st2', bufs=3)
                    nc.vector.memset(st2[:, :, NT - 1], 1.0)
                    for i in range(NT):
                        pc = 128 if i < NT - 1 else S - 128 * (NT - 1)
                        xv = xb[0:pc, i, :]
                        nc.sync.dma_start(xv, x_dram[b, 128 * i:128 * i + pc, :].bitcast(f32r))
                        # bf16 copy fused with token sum (ACT/DVE alternating)
                        xo = xb16[0:pc, 256 * i:256 * i + 256]
                        nc.vector.tensor_scalar(xo, xv, 1.0, 0.0, op0=OP.mult,
                                                op1=OP.add,
                                                accum_out=st2[0:pc, 0, i:i + 1])
                        # sum of squares from the bf16 copy (split DVE/ACT)
                        sqd = wp.tile([128, 256], bf16, name=f'sqd{s}{b}{i}', tag='sqd',
                                      bufs=3)
                        if i % 3 == 0:
                            nc.vector.scalar_tensor_tensor(sqd[0:pc, :], xo, 1.0, xo,
                                                           op0=OP.mult, op1=OP.mult,
                                                           accum_out=st2[0:pc, 1, i:i + 1])
                        else:
                            nc.scalar.activation(sqd[0:pc, :], xo, AF.Square,
                                                 accum_out=st2[0:pc, 1, i:i + 1])
                    mu = wp.tile([128, NT], f32, name=f'mu{s}{b}', tag='mu', bufs=3)
                    nc.scalar.activation(mu[:, :], st2[:, 0, :], AF.Copy, scale=1.0 / 256.0)
                    musq = wp.tile([128, NT], f32, name=f'musq{s}{b}', tag='musq', bufs=3)
                    nc.scalar.activation(musq[:, :], mu[:, :], AF.Square)
                    nc.vector.tensor_scalar(musq[:, :], musq[:, :], EPS, None,
                                            op0=OP.subtract)
                    ve = wp.tile([128, NT], f32, name=f've{s}{b}', tag='vep', bufs=3)
                    nc.vector.scalar_tensor_tensor(ve[:, :], st2[:, 1, :], 1.0 / 256.0,
                                                   musq[:, :], op0=OP.mult, op1=OP.subtract)
                    nc.vector.reciprocal(ve[:, :], ve[:, :])
                    rt = wp.tile([128, NT], f32, name=f'rt{s}{b}', tag='rt', bufs=3)
                    nc.scalar.activation(rt[:, :], ve[:, :], AF.Sqrt)
                    wall = wp.tile([128, NT, K], bf16, name=f'wall{s}{b}', tag='wall',
                                   bufs=3)
                    nc.vector.tensor_tensor(
                        wall[:, :, :],
                        maskP[:, :].rearrange("p (n k) -> p n k", k=K),
                        rt[:, :].broadcast_to([128, NT, K]),
                        op=OP.mult)
                    seg_ps = pp.tile([K, 256], f32, name=f'segps{s}{b}', tag='sm', bufs=3)
                    for i in range(NT):
                        pc = 128 if i < NT - 1 else S - 128 * (NT - 1)
                        nc.tensor.matmul(seg_ps[:, :], wall[0:pc, i, :],
                                         xb16[0:pc, 256 * i:256 * i + 256],
                                         start=(i == 0), stop=(i == NT - 1))
                    srow = wp.tile([K, 2], f32, name=f'srow{s}{b}', tag='srow', bufs=3)
                    nc.vector.reduce_sum(srow[:, 0:1], seg_ps[:, :], axis=AX.X)
                    nc.vector.tensor_scalar(srow[:, 1:2], srow[:, 0:1], -1.0 / 256.0,
                                            None, op0=OP.mult)
                    segc = wp.tile([K, 256], bf16, name=f'segc{s}{b}', tag='segc', bufs=2)
                    nc.scalar.activation(segc[:, :], seg_ps[:, :], AF.Identity,
                                         bias=srow[:, 1:2], scale=1.0)
                    tp0 = pp.tile([128, K], bf16, name=f'tp0{s}{b}', tag='smb', bufs=2)
                    nc.tensor.transpose(tp0[:, :], segc[:, 0:128], identb[0:K, 0:K])
                    nc.scalar.activation(segT0[:, cols30(b)], tp0[:, :], AF.Identity,
                                         bias=lb0[:, 0:1], scale=lw0[:, 0:1])
                    tp1 = pp.tile([128, K], bf16, name=f'tp1{s}{b}', tag='smb', bufs=2)
                    nc.tensor.transpose(tp1[:, :], segc[:, 128:256], identb[0:K, 0:K])
                    nc.scalar.activation(segT1[:, cols30(b)], tp1[:, :], AF.Identity,
                                         bias=lb1[:, 0:1], scale=lw1[:, 0:1])

                # -------- P3: MLP --------
                h1_ps = pp.tile([128, W240], f32, name=f'h1ps{s}', tag='big', bufs=3)
                nc.tensor.matmul(h1_ps[:, :], w1a[:, :], segT0[:, :],
                                 start=True, stop=False)
                nc.tensor.matmul(h1_ps[:, :], w1b[:, :], segT1[:, :],
                                 start=False, stop=True)
                (h1_sb,) = ln_feat(f'l1{s}', [h1_ps[:, :]], [b1c], [l1w], [l1b],
                                   relu=True, denom=H, out_tags=['lnA'], out_dtype=f32r)

                h2_ps = pp.tile([128, W240], f32, name=f'h2ps{s}', tag='big', bufs=3)
                nc.tensor.matmul(h2_ps[:, :], w2[:, :], h1_sb[:, :])
                (encT,) = ln_feat(f'l2{s}', [h2_ps[:, :]], [b2c], [l2w], [l2b],
                                  relu=False, denom=H, out_tags=['lnB'], out_dtype=f32r)

                # -------- P4: GCN --------
                msg_ps = pp.tile([128, W240], f32, name=f'msgps{s}', tag='big', bufs=3)
                nc.tensor.matmul(msg_ps[:, :], gw[:, :], encT[:, :])
                msgT = wp.tile([128, W240], bf16, name=f'msgT{s}', tag='msgT')
                nc.scalar.activation(msgT[:, :], msg_ps[:, :], AF.Copy, scale=0.5)

                updT_ps = pp.tile([128, W240], f32, name=f'updps{s}', tag='big', bufs=3)
                sim_ps = pp.tile([K, W240], f32, name=f'sim{s}', tag='sm', bufs=3)
                for b in range(BL):
                    nc.tensor.matmul(sim_ps[:, cols30(b)], encT[:, cols30(b)],
                                     encT[:, cols30(b)])
                z = wp.tile([K, W240], f32, name=f'z{s}', tag='z', bufs=2)
                nc.vector.tensor_tensor(z[:, :], sim_ps[:, :], tattr[:, :], op=OP.mult)
                zneg = wp.tile([K, BL], f32, name=f'zneg{s}', tag='zneg', bufs=2)
                nc.vector.reduce_max(zneg[:, :], z[:, :].rearrange("p (n k) -> p n k", k=K),
                                     axis=AX.X, negate=True)
                es = wp.tile([K, BL], f32, name=f'es{s}', tag='es', bufs=2)
                e_sb = wp.tile([K, W240], f32, name=f'e{s}', tag='esb', bufs=2)
                for b in range(BL):
                    nc.scalar.activation(e_sb[:, cols30(b)], z[:, cols30(b)], AF.Exp,
                                         bias=zneg[:, b:b + 1], scale=1.0,
                                         accum_out=es[:, b:b + 1])
                nc.vector.reciprocal(es[:, :], es[:, :])
                sm2 = wp.tile([K, W240], bf16, name=f'sm2{s}', tag='sm2', bufs=2)
                for b in range(BL):
                    nc.scalar.activation(sm2[:, cols30(b)], e_sb[:, cols30(b)], AF.Copy,
                                         scale=es[:, b:b + 1])
                tr_ps = pp.tile([K, W240], bf16, name=f'tr{s}', tag='sm', bufs=3)
                for b in range(BL):
                    nc.tensor.transpose(tr_ps[:, cols30(b)], sm2[:, cols30(b)],
                                        identb[0:K, 0:K])
                at = wp.tile([K, W240], bf16, name=f'at{s}', tag='at', bufs=2)
                nc.vector.tensor_tensor(at[:, :], tr_ps[:, :], eyefr[:, :], op=OP.add)
                for b in range(BL):
                    trm_ps = pp.tile([K, 128], bf16, name=f'trm{s}{b}', tag='smb', bufs=2)
                    nc.tensor.transpose(trm_ps[:, :], msgT[:, cols30(b)], identb[:, :])
                    msg_sb = wp.tile([K, 128], bf16, name=f'msgsb{s}{b}', tag='msgsb', bufs=2)
                    nc.scalar.activation(msg_sb[:, :], trm_ps[:, :], AF.Copy)
                    nc.tensor.matmul(updT_ps[:, cols30(b)], msg_sb[:, :],
                                     at[:, cols30(b)])

                (gc,) = ln_feat(f'gc{s}', [updT_ps[:, :]], [None], [gcw], [gcb],
                                relu=True, denom=H, out_tags=[f'gcnT{s}'], out_bufs=1,
                                out_dtype=f32r)
                gcnT[s] = gc

                # -------- P5: projection --------
                pj0_ps = pp.tile([128, W240], f32, name=f'pj0{s}', tag='big', bufs=3)
                nc.tensor.matmul(pj0_ps[:, :], pw[:, 0:128], gc[:, :])
                pj1_ps = pp.tile([128, W240], f32, name=f'pj1{s}', tag='big', bufs=3)
                nc.tensor.matmul(pj1_ps[:, :], pw[:, 128:256], gc[:, :])
                pT0, pT1 = ln_feat(f'pj{s}', [pj0_ps[:, :], pj1_ps[:, :]], [pb0, pb1],
                                   [plw0, plw1], [plb0, plb1], relu=False, denom=D,
                                   out_tags=['pT0', 'pT1'], out_dtype=bf16)

                # -------- P6: scatter + blend per batch --------
                for b in range(BL):
                    proj_sb = wp.tile([K, D], bf16, name=f'projsb{s}{b}', tag='projsb',
                                      bufs=3)
                    tq0 = pp.tile([K, 128], bf16, name=f'tq0{s}{b}', tag='smb', bufs=2)
                    nc.tensor.transpose(tq0[:, :], pT0[:, cols30(b)], identb[:, :])
                    nc.scalar.activation(proj_sb[:, 0:128], tq0[:, :], AF.Copy,
                                         scale=1.0 - ALPHA)
                    tq1 = pp.tile([K, 128], bf16, name=f'tq1{s}{b}', tag='smb', bufs=2)
                    nc.tensor.transpose(tq1[:, :], pT1[:, cols30(b)], identb[:, :])
                    nc.scalar.activation(proj_sb[:, 128:256], tq1[:, :], AF.Copy,
                                         scale=1.0 - ALPHA)
                    xb = xbs[b]
                    for i in range(NT):
                        pc = 128 if i < NT - 1 else S - 128 * (NT - 1)
                        g_ps = pp.tile([128, D], f32, name=f'g{s}{b}{i}', tag='big', bufs=3)
                        nc.tensor.matmul(g_ps[0:pc, :],
                                         maskS[:, 128 * i:128 * i + pc],
                                         proj_sb[:, :])
                        o_sb = wp.tile([128, D], f32, name=f'osb{s}{b}{i}', tag='osb',
                                       bufs=4)
                        nc.vector.scalar_tensor_tensor(o_sb[0:pc, :],
                                                       xb[0:pc, i, :], ALPHA,
                                                       g_ps[0:pc, :],
                                                       op0=OP.mult, op1=OP.add)
                        nc.gpsimd.dma_start(out_dram[b, 128 * i:128 * i + pc, :],
                                            o_sb[0:pc, :])

            # ==================== reg loss partial ====================
            pairs = [('d', 'd'), ('n', 'n'), ('e', 'e'), ('d', 'n'), ('d', 'e'), ('n', 'e')]
            acc = wp.tile([128, 6], f32, name='regacc', tag='regacc', bufs=1)
            dump = wp.tile([128, W240], f32, name='regdump', tag='regdump', bufs=2)
            for j, (a, bb) in enumerate(pairs):
                nc.vector.tensor_tensor(dump[:, :], gcnT[a][:, :], gcnT[bb][:, :],
                                        op=OP.mult)
                nc.vector.reduce_sum(acc[:, j:j + 1], dump[:, :], axis=AX.X)
            tsum = wp.tile([128, 1], f32, name='tsum', tag='tsum', bufs=1)
            nc.vector.tensor_tensor(tsum[:, :], acc[:, 0:1], acc[:, 1:2], op=OP.add)
            nc.vector.tensor_tensor(tsum[:, :], tsum[:, :], acc[:, 2:3], op=OP.add)
            usum = wp.tile([128, 1], f32, name='usum', tag='usum', bufs=1)
            nc.vector.tensor_tensor(usum[:, :], acc[:, 3:4], acc[:, 4:5], op=OP.add)
            nc.vector.tensor_tensor(usum[:, :], usum[:, :], acc[:, 5:6], op=OP.add)
            wv = wp.tile([128, 1], f32, name='wv', tag='wv', bufs=1)
            nc.vector.scalar_tensor_tensor(wv[:, :], usum[:, :], -LAM, tsum[:, :],
                                           op0=OP.mult, op1=OP.add)
            rg_ps = pp.tile([1, 1], f32, name='rgps', tag='sm', bufs=3)
            nc.tensor.matmul(rg_ps[:, :], wv[:, :], ones_col[:, :])
            rg_sb = wp.tile([1, 1], f32, name='rgsb', tag='rgsb', bufs=1)
            nc.scalar.activation(rg_sb[:, :], rg_ps[:, :], AF.Copy)
            nc.sync.dma_start(reg_out[:, :], rg_sb[:, :])

    nc.compile()
    return nc


_PROG = None


def _get_prog():
    global _PROG
    if _PROG is None:
        _PROG = _build()
    return _PROG


def _make_in_maps(inputs):
    consts = _host_consts()
    base = {}
    for name in PARAM_1D:
        base[name] = np.ascontiguousarray(
            np.asarray(inputs[name], dtype=np.float32).reshape(-1, 1))
    for name in PARAM_2D:
        base[name] = np.ascontiguousarray(np.asarray(inputs[name], dtype=np.float32))
    base.update(consts)
    in_maps = []
    for c in range(NCORES):
        m = dict(base)
        for s in ('d', 'n', 'e'):
            m[f'x_{s}'] = np.ascontiguousarray(
                np.asarray(inputs[f'x_{s}'], dtype=np.float32)[c * BL:(c + 1) * BL])
        in_maps.append(m)
    return in_maps


def _run(in_maps, trace=False):
    from concourse.bass_utils import run_bass_kernel_spmd
    nc = _get_prog()
    return run_bass_kernel_spmd(nc, in_maps, core_ids=list(range(NCORES)), trace=trace)


def kernel(**inputs):
    in_maps = _make_in_maps(inputs)
    res = _run(in_maps, trace=False)
    outs = res.results
    out_d = np.concatenate([outs[c]['out_d'] for c in range(NCORES)], axis=0)
    out_n = np.concatenate([outs[c]['out_n'] for c in range(NCORES)], axis=0)
    out_e = np.concatenate([outs[c]['out_e'] for c in range(NCORES)], axis=0)
    reg = np.float32(sum(float(outs[c]['reg_partial'][0, 0]) for c in range(NCORES)) / B)
    return out_d, out_n, out_e, reg
